# revision 1
# baseline (speedup 1.0000x reference)
"""AttentionBlock (GroupNorm + 1x1-conv QKV + spatial attention + 1x1-conv out
+ skip) on 8 Trainium2 NeuronCores.

Sharding: data-parallel over batch. B=16 -> 2 batches per core, weights
replicated, no collectives. Each core runs the same NEFF on its own batch
slice; the host gathers by concatenation.

Layouts on chip (partition dim first):
  channels  c = 128*ct + p   (ct in 0..3)   -> tiles [128, 4, *]
  spatial   n = 128*nb + p   (nb in 0..7)
  x, xn, q, k   [128, 4, 1024]  ([c_part, ct, n])
  vT            [128, 8, 512]   ([m_part, mb, c])   (v computed transposed)
  attnT (half)  [128, 8, 512]   ([m_part, mb, n_half])
  out_sb        [128, 4, 1024]  ([c_part, ct, n])

GroupNorm cross-partition reduction/broadcast is done with tiny indicator
matmuls (K=128 group-sum, K=8 broadcast). Softmax row stats live on the
partition dim, so they are plain free-dim reduces. The attention matrix is
transposed 128x128-blockwise on the PE (identity matmul) so the AV product
can contract over the spatial dim on the partition axis.
"""

import numpy as np

B, C, H, W = 16, 512, 32, 32
HW = H * W            # 1024
BL = 2                # batches per core
NCORES = 8
CT = C // 128         # 4 channel chunks
NBLK = HW // 128      # 8 spatial blocks
GSIZE = 16            # channels per group
GSLOT = 128 // GSIZE  # 8 groups per channel chunk
CNT = GSIZE * HW      # elements per group (16384)
EPS = 1e-5
INVSQ = float(1.0 / np.sqrt(np.float32(C)))

# Matmul operand precision. "bf16": 2 elem/cycle PE streaming (fastest),
# "f32r": single-pass fp32 (e8m11, 1 elem/cycle), "f32": exact (1/4 rate).
import os
COMPUTE = os.environ.get("K_COMPUTE", "bf16")
# fp8 DoubleRow for the attention@V contraction (2 MACs/cell/cycle).
AV_FP8 = os.environ.get("K_AV_FP8", "0") == "1"
ATT_SCALE = 64.0

_CACHE = {}


def _build_program():
    import concourse.bacc as bacc
    import concourse.tile as tile
    from concourse import mybir
    from concourse.tile_rust import add_dep_helper

    F32 = mybir.dt.float32
    F32R = mybir.dt.float32r
    Alu = mybir.AluOpType
    Act = mybir.ActivationFunctionType
    Ax = mybir.AxisListType

    CDT = {"f32r": F32R, "bf16": mybir.dt.bfloat16, "f32": F32}[COMPUTE]

    nc = bacc.Bacc("TRN2", target_bir_lowering=False, debug=False)

    x_d = nc.dram_tensor("x", [BL, C, HW], F32, kind="ExternalInput")
    win_d = nc.dram_tensor("w_inT", [C, 3 * C], CDT, kind="ExternalInput")
    wout_d = nc.dram_tensor("w_outT", [C, C], CDT, kind="ExternalInput")
    gam_d = nc.dram_tensor("gamma_t", [128, CT], F32, kind="ExternalInput")
    bet_d = nc.dram_tensor("beta_t", [128, CT], F32, kind="ExternalInput")
    bqk_d = nc.dram_tensor("b_qk", [128, 2 * CT], F32, kind="ExternalInput")
    bv_d = nc.dram_tensor("b_v", [128, CT], F32, kind="ExternalInput")
    bo_d = nc.dram_tensor("b_out_t", [128, CT], F32, kind="ExternalInput")
    idn_d = nc.dram_tensor("ident", [128, 128], CDT, kind="ExternalInput")
    idn_dn_d = nc.dram_tensor("ind_dn", [128, GSLOT], F32, kind="ExternalInput")
    idn_up_d = nc.dram_tensor("ind_up", [GSLOT, 128], F32, kind="ExternalInput")
    out_d = nc.dram_tensor("out", [BL, C, HW], F32, kind="ExternalOutput")

    with tile.TileContext(nc) as tc:
        with (
            tc.tile_pool(name="consts", bufs=1) as cp,
            tc.tile_pool(name="work", bufs=1) as wp,
            tc.tile_pool(name="psum", bufs=6, space="PSUM") as pp,
        ):
            # ---- PE warm-up: the HAM clock gate holds the PE at 1.2 GHz
            # until it sees ~3.4us of sustained matmul activity. Burn idle
            # lead-in time (DMA + GroupNorm) on throwaway matmuls over a
            # memset tile so the real qkv/scores stream starts at 2.4 GHz.
            warm = cp.tile([128, 512], CDT, name="warm", tag="warm")
            nc.gpsimd.memset(warm[:], 1.0)
            warm_ps = pp.tile([128, 512], F32, name="warm_ps", tag="gstat", bufs=1)

            def warmup(n):
                for _ in range(n):
                    nc.tensor.matmul(warm_ps[:], warm[:, 0:128], warm[:], start=True, stop=True)

            warmup(20)

            # ---- small constants on the SWDGE queue ----
            ident = cp.tile([128, 128], CDT, name="ident", tag="ident")
            nc.gpsimd.dma_start(ident[:], idn_d[:])
            ind_dn = cp.tile([128, GSLOT], F32, name="ind_dn", tag="ind_dn")
            nc.gpsimd.dma_start(ind_dn[:], idn_dn_d[:])
            ind_up = cp.tile([GSLOT, 128], F32, name="ind_up", tag="ind_up")
            nc.gpsimd.dma_start(ind_up[:], idn_up_d[:])
            gam = cp.tile([128, CT], F32, name="gam", tag="gam")
            nc.gpsimd.dma_start(gam[:], gam_d[:])
            bet = cp.tile([128, CT], F32, name="bet", tag="bet")
            nc.gpsimd.dma_start(bet[:], bet_d[:])
            b_qk = cp.tile([128, 2 * CT], F32, name="b_qk", tag="b_qk")
            nc.gpsimd.dma_start(b_qk[:], bqk_d[:])
            b_v = cp.tile([128, CT], F32, name="b_v", tag="b_v")
            nc.gpsimd.dma_start(b_v[:], bv_d[:])
            b_o = cp.tile([128, CT], F32, name="b_o", tag="b_o")
            nc.gpsimd.dma_start(b_o[:], bo_d[:])

            def load_weights(part):
                # interleaved with batch-0's x chunks on the sync queue: the
                # DMA engine rings serialize same-queue transfers, so pieces
                # land progressively in the order the PE will need them
                if part == 0:
                    self_w_in = cp.tile([128, CT, 3 * C], CDT, name="w_in", tag="w_in")
                    win_r = win_d.rearrange("(kc p) o -> p kc o", p=128)
                    nc.sync.dma_start(self_w_in[:, :, 0 : 2 * C], win_r[:, :, 0 : 2 * C])
                    return self_w_in
                w_in = st[0]["w_in"]
                win_r = win_d.rearrange("(kc p) o -> p kc o", p=128)
                nc.sync.dma_start(w_in[:, :, 2 * C : 3 * C], win_r[:, :, 2 * C : 3 * C])
                w_out = cp.tile([128, CT, C], CDT, name="w_out", tag="w_out")
                nc.sync.dma_start(w_out[:], wout_d.rearrange("(kc p) o -> p kc o", p=128))
                return w_out

            # Per-batch state, filled by the phase emitters below. The two
            # batches are software-pipelined: emission order sets scheduler
            # priority, and the PE executes its queue in order, so batch b+1
            # "filler" work is emitted right after the batch-b work that
            # precedes each PE gap.
            st = [dict() for _ in range(BL)]

            def load_x(b, after=None):
                s = st[b]
                s["x"] = wp.tile([128, CT, HW], F32, name=f"x{b}", tag="x", bufs=2)
                x_r = x_d[b].rearrange("(ct p) n -> p ct n", p=128)
                for ct in range(CT):
                    dma = nc.sync.dma_start(s["x"][:, ct, :], x_r[:, ct, :])
                    if after is not None:
                        # keep this transfer off the HBM port until the
                        # earlier batch's input is consumed
                        add_dep_helper(dma.ins, after, reason="stagger x loads")

            def gn_stats_ct(b, ct):
                # per-partition sum (DVE) and sum of squares (ACT, fused
                # accumulator); GroupNorm is separable per channel chunk
                s = st[b]
                if "ssum" not in s:
                    s["ssum"] = wp.tile([128, CT, 2], F32, name=f"ssum{b}", tag="ssum", bufs=2)
                scr = wp.tile([128, HW], F32, name=f"scr{b}_{ct}", tag="scr", bufs=2)
                nc.vector.tensor_reduce(
                    out=s["ssum"][:, ct, 0:1], in_=s["x"][:, ct, :], axis=Ax.X, op=Alu.add
                )
                nc.scalar.activation(
                    scr[:], s["x"][:, ct, :], Act.Square,
                    accum_out=s["ssum"][:, ct, 1:2],
                )

            def gn_stats(b):
                for ct in range(CT):
                    gn_stats_ct(b, ct)

            def gn_apply_ct(b, ct):
                # group sums across partitions (tiny PE matmul against the
                # indicator), mean/rstd chain, broadcast back, fused apply
                s = st[b]
                if "xn" not in s:
                    s["xn"] = wp.tile([128, CT, HW], CDT, name=f"xn{b}", tag="xn", bufs=2)
                    s["ab"] = wp.tile([128, 2 * CT], F32, name=f"ab{b}", tag="ab", bufs=2)
                ab = s["ab"]
                if True:
                    ps_g = pp.tile([GSLOT, 2], F32, name=f"psg{b}_{ct}", tag="gstat", bufs=1)
                    nc.tensor.matmul(ps_g[:], ind_dn[:], s["ssum"][:, ct, :], start=True, stop=True)
                    m_r = wp.tile([GSLOT, 2], F32, name=f"mr{b}_{ct}", tag="mr", bufs=4)
                    t2 = wp.tile([GSLOT, 2], F32, name=f"t2{b}_{ct}", tag="t2", bufs=4)
                    nc.scalar.mul(m_r[:, 0:1], ps_g[:, 0:1], 1.0 / CNT)     # mean
                    nc.scalar.mul(t2[:, 0:1], ps_g[:, 1:2], 1.0 / CNT)      # E[x^2]
                    nc.vector.tensor_mul(t2[:, 1:2], m_r[:, 0:1], m_r[:, 0:1])
                    nc.vector.tensor_sub(t2[:, 0:1], t2[:, 0:1], t2[:, 1:2])
                    nc.vector.tensor_scalar_add(t2[:, 0:1], t2[:, 0:1], EPS)
                    nc.scalar.activation(t2[:, 0:1], t2[:, 0:1], Act.Sqrt)
                    nc.vector.reciprocal(m_r[:, 1:2], t2[:, 0:1])           # rstd
                    ps_bc = pp.tile([128, 2], F32, name=f"psbc{b}_{ct}", tag="gbc", bufs=1)
                    nc.tensor.matmul(ps_bc[:], ind_up[:], m_r[:], start=True, stop=True)
                    nc.vector.tensor_mul(ab[:, ct : ct + 1], ps_bc[:, 1:2], gam[:, ct : ct + 1])
                    nc.vector.tensor_mul(ab[:, CT + ct : CT + ct + 1], ps_bc[:, 0:1], ab[:, ct : ct + 1])
                    nc.vector.tensor_sub(ab[:, CT + ct : CT + ct + 1], bet[:, ct : ct + 1], ab[:, CT + ct : CT + ct + 1])
                    ap_i = nc.gpsimd.tensor_scalar(
                        out=s["xn"][:, ct, :], in0=s["x"][:, ct, :],
                        scalar1=ab[:, ct : ct + 1], scalar2=ab[:, CT + ct : CT + ct + 1],
                        op0=Alu.mult, op1=Alu.add,
                    )
                    s["last_apply"] = ap_i.ins

            def gn_apply(b):
                for ct in range(CT):
                    gn_apply_ct(b, ct)

            def qkv_wave_mm(b, wave, kcs):
                # one kc-slab of a q/k wave; accumulation groups stay open
                # across calls so GN chain matmuls can interleave
                s = st[b]
                if "q" not in s:
                    s["q"] = wp.tile([128, CT, HW], CDT, name=f"q{b}", tag="q", bufs=2)
                    s["k"] = wp.tile([128, CT, HW], CDT, name=f"k{b}", tag="k", bufs=2)
                key = ("wv", tuple(wave))
                if key not in s:
                    s[key] = [pp.tile([128, 512], F32, name=f"qk{b}_{j}_{nh}", tag="mm")
                              for (j, nh) in wave]
                for kc in kcs:
                    for ps, (j, nh) in zip(s[key], wave):
                        nc.tensor.matmul(
                            ps[:],
                            w_in[:, kc, j * 128 : (j + 1) * 128],
                            s["xn"][:, kc, nh * 512 : (nh + 1) * 512],
                            start=(kc == 0), stop=(kc == CT - 1),
                        )
                if kcs[-1] == CT - 1:
                    for gi, (ps, (j, nh)) in enumerate(zip(s[key], wave)):
                        dst = s["q"] if j < CT else s["k"]
                        sl = dst[:, j % CT, nh * 512 : (nh + 1) * 512]
                        if gi % 2 == 0:
                            nc.scalar.activation(
                                sl, ps[:], Act.Identity, bias=b_qk[:, j : j + 1]
                            )
                        else:
                            nc.vector.tensor_scalar_add(sl, ps[:], b_qk[:, j : j + 1])

            def qkv_qk(b, waves):
                for wave in waves:
                    qkv_wave_mm(b, wave, list(range(CT)))

            def qkv_v(b):
                # v computed transposed ([N, C]) by swapping matmul operands
                s = st[b]
                xn = s["xn"]
                vdt = mybir.dt.float8e4 if AV_FP8 else CDT
                s["vt"] = wp.tile([128, NBLK, C], vdt, name=f"vt{b}", tag="vt", bufs=2)
                for wave in ([0, 1, 2, 3], [4, 5, 6, 7]):
                    pss = [pp.tile([128, 512], F32, name=f"vt{b}_{mb}", tag="mm")
                           for mb in wave]
                    for kc in range(CT):
                        for ps, mb in zip(pss, wave):
                            nc.tensor.matmul(
                                ps[:],
                                xn[:, kc, mb * 128 : (mb + 1) * 128],
                                w_in[:, kc, 2 * C : 3 * C],
                                start=(kc == 0), stop=(kc == CT - 1),
                            )
                    for ps, mb in zip(pss, wave):
                        nc.vector.tensor_copy(s["vt"][:, mb, :], ps[:])

            def scores_nb(b, h, j):
                # scores for query rows [nb*128, (nb+1)*128) + softmax
                s = st[b]
                if f"sblk{h}" not in s:
                    s[f"sblk{h}"] = []
                if True:
                    nb = h * 4 + j
                    ps_s = []
                    for mh in range(2):
                        ps = pp.tile([128, 512], F32, name=f"sc{b}_{nb}_{mh}", tag="mm")
                        for kc in range(CT):
                            nc.tensor.matmul(
                                ps[:],
                                s["q"][:, kc, nb * 128 : (nb + 1) * 128],
                                s["k"][:, kc, mh * 512 : (mh + 1) * 512],
                                start=(kc == 0), stop=(kc == CT - 1),
                            )
                        ps_s.append(ps)
                    rs = wp.tile([128, 3], F32, name=f"rs{b}_{nb}", tag="rs", bufs=4)
                    sblk = wp.tile([128, HW], CDT, name=f"sb{b}_{nb}", tag="sblk", bufs=8)
                    rsum = rs
                    # scores/sqrt(C) is ~N(0,1) here, so exp() needs no max
                    # subtraction (softmax is shift-invariant; fp32 exp is
                    # exact for this range)
                    for mh in range(2):
                        nc.scalar.activation(
                            sblk[:, mh * 512 : (mh + 1) * 512], ps_s[mh][:],
                            Act.Exp, bias=0.0, scale=INVSQ,
                            accum_out=rsum[:, mh : mh + 1],
                        )
                    nc.vector.tensor_add(rsum[:, 2:3], rsum[:, 0:1], rsum[:, 1:2])
                    nc.vector.reciprocal(rsum[:, 0:1], rsum[:, 2:3])
                    nc.vector.tensor_scalar_mul(sblk[:], sblk[:], rsum[:, 0:1])
                    s[f"sblk{h}"].append(sblk)

            def scores_half(b, h):
                for j in range(4):
                    scores_nb(b, h, j)

            def trans_av(b, h):
                # PE-transpose the 4 softmaxed row blocks, then contract with
                # vT over the key dim (partition axis)
                s = st[b]
                adt = mybir.dt.float8e4 if AV_FP8 else CDT
                att = wp.tile([128, NBLK, 512], adt, name=f"att{b}_{h}", tag="att", bufs=2)
                sblks = s[f"sblk{h}"]
                for mb in range(NBLK):
                    ps = pp.tile([128, 512], CDT, name=f"tr{b}_{h}_{mb}", tag="mm")
                    for j in range(4):
                        nc.tensor.transpose(
                            ps[:, j * 128 : (j + 1) * 128],
                            sblks[j][:, mb * 128 : (mb + 1) * 128],
                            ident[:],
                        )
                    if AV_FP8:
                        # attn weights are <=~0.05; scale into fp8e4m3's
                        # normal range (undone in the AV evacuation)
                        nc.vector.tensor_scalar_mul(att[:, mb, :], ps[:], ATT_SCALE)
                    elif mb % 2 == 0:
                        nc.vector.tensor_copy(att[:, mb, :], ps[:])
                    else:
                        nc.scalar.copy(att[:, mb, :], ps[:])
                if "o" not in s:
                    s["o"] = wp.tile([128, CT, HW], CDT, name=f"o{b}", tag="osb", bufs=2)
                for ct in range(CT):
                    ps = pp.tile([128, 512], F32, name=f"av{b}_{h}_{ct}", tag="mm")
                    if AV_FP8:
                        for mp in range(NBLK // 2):   # mb pairs, 2 packed/cell
                            nc.tensor.matmul(
                                ps[:],
                                s["vt"][:, 2 * mp : 2 * mp + 2, ct * 128 : (ct + 1) * 128],
                                att[:, 2 * mp : 2 * mp + 2, :],
                                start=(mp == 0), stop=(mp == NBLK // 2 - 1),
                                perf_mode=mybir.MatmulPerfMode.DoubleRow,
                            )
                    else:
                        for mb in range(NBLK):
                            nc.tensor.matmul(
                                ps[:],
                                s["vt"][:, mb, ct * 128 : (ct + 1) * 128],
                                att[:, mb, :],
                                start=(mb == 0), stop=(mb == NBLK - 1),
                            )
                    osl = s["o"][:, ct, h * 512 : (h + 1) * 512]
                    if AV_FP8:
                        nc.scalar.activation(
                            osl, ps[:], Act.Identity, bias=b_v[:, ct : ct + 1],
                            scale=1.0 / ATT_SCALE,
                        )
                    elif ct % 2 == 0:
                        nc.scalar.activation(
                            osl, ps[:], Act.Identity, bias=b_v[:, ct : ct + 1]
                        )
                    else:
                        nc.vector.tensor_scalar_add(osl, ps[:], b_v[:, ct : ct + 1])

            def proj_half(b, nh):
                # output projection for one n-half; skip + out-bias fused
                # into the PSUM evacuation, written back over x, streamed out
                s = st[b]
                out_r = out_d[b].rearrange("(ct p) n -> p ct n", p=128)
                for ct in range(CT):
                    ps = pp.tile([128, 512], F32, name=f"pj{b}_{ct}_{nh}", tag="mm")
                    for kc in range(CT):
                        nc.tensor.matmul(
                            ps[:],
                            w_out[:, kc, ct * 128 : (ct + 1) * 128],
                            s["o"][:, kc, nh * 512 : (nh + 1) * 512],
                            start=(kc == 0), stop=(kc == CT - 1),
                        )
                    sl = s["x"][:, ct, nh * 512 : (nh + 1) * 512]
                    nc.vector.scalar_tensor_tensor(
                        out=sl, in0=ps[:], scalar=b_o[:, ct : ct + 1], in1=sl,
                        op0=Alu.add, op1=Alu.add,
                    )
                    nc.sync.dma_start(out_r[:, ct, nh * 512 : (nh + 1) * 512], sl)

            # ---- software-pipelined emission across the two batches ----
            W_Q0 = [(0, 0), (1, 0), (2, 0), (3, 0)]
            W_K0 = [(4, 0), (5, 0), (6, 0), (7, 0)]
            W_Q1 = [(0, 1), (1, 1), (2, 1), (3, 1)]
            W_K1 = [(4, 1), (5, 1), (6, 1), (7, 1)]
            def load_x_ct(b, ct):
                s = st[b]
                if "x" not in s:
                    s["x"] = wp.tile([128, CT, HW], F32, name=f"x{b}", tag="x", bufs=2)
                x_r = x_d[b].rearrange("(ct p) n -> p ct n", p=128)
                nc.sync.dma_start(s["x"][:, ct, :], x_r[:, ct, :])

            load_x_ct(0, 0); load_x_ct(0, 1)
            st[0]["w_in"] = w_in = load_weights(0)      # q+k columns
            load_x_ct(0, 2); load_x_ct(0, 3)
            w_out = load_weights(1)                      # v columns + w_out
            gn_stats(0); gn_apply(0)
            qkv_qk(0, [W_Q0, W_K0, W_K1])   # enough for h0 scores
            load_x(1)                       # sync queue, behind weights
            scores_nb(0, 0, 0); scores_nb(0, 0, 1)
            qkv_qk(0, [W_Q1])
            scores_nb(0, 0, 2); scores_nb(0, 0, 3)
            gn_stats(1)
            qkv_v(0)
            gn_apply(1)
            trans_av(0, 0)
            scores_half(0, 1)
            qkv_qk(1, [W_Q0, W_K0, W_K1])
            trans_av(0, 1)
            proj_half(0, 0)
            scores_nb(1, 0, 0); scores_nb(1, 0, 1)
            qkv_qk(1, [W_Q1])
            scores_nb(1, 0, 2); scores_nb(1, 0, 3)
            qkv_v(1)
            proj_half(0, 1)
            trans_av(1, 0)
            scores_half(1, 1)
            proj_half(1, 0)
            trans_av(1, 1)
            proj_half(1, 1)

    nc.compile()
    return nc


def _get_program():
    key = (COMPUTE, AV_FP8)
    if key not in _CACHE:
        _CACHE[key] = _build_program()
    return _CACHE[key]


def _to_compute(a):
    """Convert host fp32 weights to the matmul compute format."""
    if COMPUTE == "bf16":
        import ml_dtypes
        return np.ascontiguousarray(np.asarray(a, dtype=np.float32).astype(ml_dtypes.bfloat16))
    if COMPUTE == "f32r":
        # e8m11 round-to-nearest-even, low 12 bits zero (PE fp32r format)
        bits = np.ascontiguousarray(a, dtype=np.float32).view(np.uint32)
        lsb = (bits >> 12) & 1
        out = ((bits + 0x7FF + lsb) & np.uint32(0xFFFFF000)).view(np.float32)
        return np.ascontiguousarray(out)
    return np.ascontiguousarray(a, dtype=np.float32)


def _make_in_maps(x, gamma, beta, w_in, b_in, w_out, b_out):
    x = np.ascontiguousarray(x.reshape(B, C, HW), dtype=np.float32)
    consts = {
        "w_inT": _to_compute(w_in.T),
        "w_outT": _to_compute(w_out.T),
        "gamma_t": np.ascontiguousarray(gamma.reshape(CT, 128).T, dtype=np.float32),
        "beta_t": np.ascontiguousarray(beta.reshape(CT, 128).T, dtype=np.float32),
        "b_qk": np.ascontiguousarray(b_in[: 2 * C].reshape(2 * CT, 128).T, dtype=np.float32),
        "b_v": np.ascontiguousarray(b_in[2 * C :].reshape(CT, 128).T, dtype=np.float32),
        "b_out_t": np.ascontiguousarray(b_out.reshape(CT, 128).T, dtype=np.float32),
        "ident": _to_compute(np.eye(128, dtype=np.float32)),
        "ind_dn": (np.arange(128)[:, None] // GSIZE == np.arange(GSLOT)[None, :]).astype(np.float32),
        "ind_up": (np.arange(GSLOT)[:, None] == np.arange(128)[None, :] // GSIZE).astype(np.float32),
    }
    return [
        {"x": x[c * BL : (c + 1) * BL], **consts}
        for c in range(NCORES)
    ]


def run(inputs, trace=False):
    """Run on 8 cores; returns (output [B,C,H,W], BassKernelResults)."""
    from concourse.bass_utils import run_bass_kernel_spmd

    nc = _get_program()
    in_maps = _make_in_maps(**inputs)
    res = run_bass_kernel_spmd(nc, in_maps, core_ids=list(range(NCORES)), trace=trace)
    out = np.concatenate([res.results[i]["out"] for i in range(NCORES)], axis=0)
    return out.reshape(B, C, H, W).astype(np.float32), res


def kernel(**inputs) -> np.ndarray:
    out, _ = run(inputs)
    return out



# revision 6
# speedup vs baseline: 1.4133x; 1.4133x over previous
"""AttentionBlock (GroupNorm + 1x1-conv QKV + spatial attention + 1x1-conv out
+ skip) on 8 Trainium2 NeuronCores.

Sharding: data-parallel over batch. B=16 -> 2 batches per core, weights
replicated, no collectives. Each core runs the same NEFF on its own batch
slice; the host gathers by concatenation.

v2 redesign (vs the q/k/v baseline):
  * Host folds the 1x1 convs:  M1 = W_q^T W_k  and  M2 = W_o W_v, so
        scores = xn^T M1 xn          (one projection t = M1 xn instead of q,k)
        out    = (M2 xn) attn^T + (W_o b_v + b_o)   (no separate v / proj_out)
    The bias fold is exact because softmax rows sum to 1.
  * Scores are computed TRANSPOSED ([key m on partitions, query n free]):
    kills all PE transposes and the attn normalization pass. The softmax
    denominator Z comes from a ones-stationary matmul over exp(scores^T);
    normalization happens once, fused into the output evacuation
    (out = outU * (1/Z)[n] + bias + skip).
  * All five big matmul groups run fp8e4 DoubleRow (2 MACs/cell/cycle);
    exp() output is biased into fp8 range (any fixed scale cancels in Z).

Layouts on chip (partition dim first):
  channels  c = 128*ct + p   (ct in 0..3)
  spatial   n = 128*mb + p   (mb in 0..7)
  x             [128, 4, 1024]  f32   ([c_part, ct, n])
  xn, t         [128, 4, 1024]  fp8   (t indexed [c_q, m])
  ut            [128, 8, 512]   fp8   ([m_part, mb, c_out])
  E = exp(S^T)  [128, 8, 1024]  fp8   ([m_part, mb, n])
  zr = 1/Z      [128, 2, 512]   f32   (Z broadcast over partitions)

GroupNorm cross-partition reduction/broadcast is done with tiny indicator
matmuls (K=128 group-sum, K=8 broadcast), as in the baseline.
"""

import os
import numpy as np

B, C, H, W = 16, 512, 32, 32
HW = H * W            # 1024
BL = 2                # batches per core
NCORES = 8
CT = C // 128         # 4 channel chunks
NBLK = HW // 128      # 8 spatial blocks
GSIZE = 16            # channels per group
GSLOT = 128 // GSIZE  # 8 groups per channel chunk
CNT = GSIZE * HW      # elements per group (16384)
EPS = 1e-5
INVSQ = float(1.0 / np.sqrt(np.float32(C)))
# exp(score/sqrt(C) - SHIFT): keeps exp output in fp8e4m3's range
# (max |score/sqrt(C)| ~ 5.6 -> exp <= ~60 << 240). The scale e^-SHIFT
# cancels exactly in the Z normalization.
SHIFT = float(os.environ.get("K_SHIFT", "1.5"))
# "fp8": DoubleRow fp8 for all big matmuls. "bf16": same structure, bf16.
V2DT = os.environ.get("K_V2DT", "fp8")

_CACHE = {}


def _build_program(need_rx):
    import concourse.bacc as bacc
    import concourse.tile as tile
    from concourse import mybir
    from concourse.tile_rust import add_dep_helper

    F32 = mybir.dt.float32
    Alu = mybir.AluOpType
    Act = mybir.ActivationFunctionType
    Ax = mybir.AxisListType
    BF16 = mybir.dt.bfloat16
    FP8 = V2DT == "fp8"
    CDT = mybir.dt.float8e4 if FP8 else BF16
    DR = mybir.MatmulPerfMode.DoubleRow if FP8 else None
    # kc contraction steps: pairs under DoubleRow, singles otherwise
    KSTEP = 2 if FP8 else 1
    NK = CT // KSTEP
    NM = NBLK // KSTEP

    nc = bacc.Bacc("TRN2", target_bir_lowering=False, debug=False)

    x_d = nc.dram_tensor("x", [BL, C, HW], F32, kind="ExternalInput")
    m1_d = nc.dram_tensor("m1t", [C, C], CDT, kind="ExternalInput")
    m2_d = nc.dram_tensor("m2t", [C, C], CDT, kind="ExternalInput")
    gam_d = nc.dram_tensor("gamma_t", [128, CT], F32, kind="ExternalInput")
    bet_d = nc.dram_tensor("beta_t", [128, CT], F32, kind="ExternalInput")
    c1_d = nc.dram_tensor("c1_t", [128, CT], F32, kind="ExternalInput")
    bf_d = nc.dram_tensor("bf_t", [128, CT], F32, kind="ExternalInput")
    idn_dn_d = nc.dram_tensor("ind_dn", [128, GSLOT], F32, kind="ExternalInput")
    idn_up_d = nc.dram_tensor("ind_up", [GSLOT, 128], F32, kind="ExternalInput")
    if need_rx:
        wr_d = nc.dram_tensor("wr_t", [128, CT], CDT, kind="ExternalInput")
    out_d = nc.dram_tensor("out", [BL, C, HW], F32, kind="ExternalOutput")

    with tile.TileContext(nc) as tc:
        with (
            tc.tile_pool(name="consts", bufs=1) as cp,
            tc.tile_pool(name="work", bufs=1) as wp,
            tc.tile_pool(name="psum", bufs=1, space="PSUM") as pp,
        ):
            # ---- PE warm-up: the HAM clock gate holds the PE at 1.2 GHz
            # until it sees ~3.4us of sustained matmul activity. Burn the
            # DMA + GroupNorm lead-in on throwaway matmuls.
            warm = cp.tile([128, 512], BF16, name="warm", tag="warm")
            nc.gpsimd.memset(warm[:], 1.0)
            warm_ps = pp.tile([128, 512], F32, name="warm_ps", tag="gstat", bufs=1)

            def warmup(n):
                for _ in range(n):
                    nc.tensor.matmul(warm_ps[:], warm[:, 0:128], warm[:], start=True, stop=True)

            warmup(20)

            # ---- small constants on the SWDGE queue ----
            ind_dn = cp.tile([128, GSLOT], F32, name="ind_dn", tag="ind_dn")
            nc.gpsimd.dma_start(ind_dn[:], idn_dn_d[:])
            ind_up = cp.tile([GSLOT, 128], F32, name="ind_up", tag="ind_up")
            nc.gpsimd.dma_start(ind_up[:], idn_up_d[:])
            gam = cp.tile([128, CT], F32, name="gam", tag="gam")
            nc.gpsimd.dma_start(gam[:], gam_d[:])
            bet = cp.tile([128, CT], F32, name="bet", tag="bet")
            nc.gpsimd.dma_start(bet[:], bet_d[:])
            c1 = cp.tile([128, CT], F32, name="c1", tag="c1")
            nc.gpsimd.dma_start(c1[:], c1_d[:])
            b_f = cp.tile([128, CT], F32, name="b_f", tag="b_f")
            nc.gpsimd.dma_start(b_f[:], bf_d[:])
            if need_rx:
                wr = cp.tile([128, CT], CDT, name="wr", tag="wr")
                nc.gpsimd.dma_start(wr[:], wr_d[:])
            # device-built constants
            ones_dr = cp.tile([128, KSTEP, 16], CDT, name="ones_dr", tag="ones_dr")
            nc.gpsimd.memset(ones_dr[:], 1.0)
            ones_bc = cp.tile([1, 128], BF16, name="ones_bc", tag="ones_bc")
            nc.gpsimd.memset(ones_bc[:], 1.0)
            ebias = cp.tile([128, 1], F32, name="ebias", tag="ebias")
            nc.gpsimd.memset(ebias[:], -SHIFT)

            def load_weights(part):
                # interleaved with batch-0's x chunks on the sync queue so
                # pieces land progressively in the order the PE needs them
                if part == 0:
                    m1 = cp.tile([128, CT, C], CDT, name="m1", tag="m1")
                    nc.sync.dma_start(m1[:], m1_d.rearrange("(kc p) o -> p kc o", p=128))
                    return m1
                m2 = cp.tile([128, CT, C], CDT, name="m2", tag="m2")
                nc.sync.dma_start(m2[:], m2_d.rearrange("(kc p) o -> p kc o", p=128))
                return m2

            st = [dict() for _ in range(BL)]

            def load_x_ct(b, ct, after=None):
                s = st[b]
                if "x" not in s:
                    s["x"] = wp.tile([128, CT, HW], F32, name=f"x{b}", tag="x", bufs=2)
                x_r = x_d[b].rearrange("(ct p) n -> p ct n", p=128)
                dma = nc.sync.dma_start(s["x"][:, ct, :], x_r[:, ct, :])
                if after is not None:
                    add_dep_helper(dma.ins, after, reason="stagger x loads")
                return dma

            def gn_stats(b):
                # per-partition sum (DVE) and sum of squares (ACT, fused
                # accumulator); GroupNorm is separable per channel chunk
                s = st[b]
                s["ssum"] = wp.tile([128, CT, 2], F32, name=f"ssum{b}", tag="ssum", bufs=2)
                scr = wp.tile([128, HW], F32, name=f"scr{b}", tag="scr", bufs=2)
                for ct in range(CT):
                    nc.vector.tensor_reduce(
                        out=s["ssum"][:, ct, 0:1], in_=s["x"][:, ct, :], axis=Ax.X, op=Alu.add
                    )
                    nc.scalar.activation(
                        scr[:], s["x"][:, ct, :], Act.Square,
                        accum_out=s["ssum"][:, ct, 1:2],
                    )

            def gn_apply(b):
                # group sums across partitions (tiny PE matmul against the
                # indicator), mean/rstd chain, broadcast back, fused apply
                # writing xn in the matmul compute dtype
                s = st[b]
                s["xn"] = wp.tile([128, CT, HW], CDT, name=f"xn{b}", tag="xn", bufs=2)
                s["ab"] = wp.tile([128, 2 * CT], F32, name=f"ab{b}", tag="ab", bufs=2)
                ab = s["ab"]
                for ct in range(CT):
                    ps_g = pp.tile([GSLOT, 2], F32, name=f"psg{b}_{ct}", tag="gstat", bufs=1)
                    nc.tensor.matmul(ps_g[:], ind_dn[:], s["ssum"][:, ct, :], start=True, stop=True)
                    m_r = wp.tile([GSLOT, 2], F32, name=f"mr{b}_{ct}", tag="mr", bufs=4)
                    t2 = wp.tile([GSLOT, 2], F32, name=f"t2{b}_{ct}", tag="t2", bufs=4)
                    nc.scalar.mul(m_r[:, 0:1], ps_g[:, 0:1], 1.0 / CNT)     # mean
                    nc.scalar.mul(t2[:, 0:1], ps_g[:, 1:2], 1.0 / CNT)      # E[x^2]
                    nc.vector.tensor_mul(t2[:, 1:2], m_r[:, 0:1], m_r[:, 0:1])
                    nc.vector.tensor_sub(t2[:, 0:1], t2[:, 0:1], t2[:, 1:2])
                    nc.vector.tensor_scalar_add(t2[:, 0:1], t2[:, 0:1], EPS)
                    nc.scalar.activation(t2[:, 0:1], t2[:, 0:1], Act.Sqrt)
                    nc.vector.reciprocal(m_r[:, 1:2], t2[:, 0:1])           # rstd
                    ps_bc = pp.tile([128, 2], F32, name=f"psbc{b}_{ct}", tag="gbc", bufs=1)
                    nc.tensor.matmul(ps_bc[:], ind_up[:], m_r[:], start=True, stop=True)
                    nc.vector.tensor_mul(ab[:, ct : ct + 1], ps_bc[:, 1:2], gam[:, ct : ct + 1])
                    nc.vector.tensor_mul(ab[:, CT + ct : CT + ct + 1], ps_bc[:, 0:1], ab[:, ct : ct + 1])
                    nc.vector.tensor_sub(ab[:, CT + ct : CT + ct + 1], bet[:, ct : ct + 1], ab[:, CT + ct : CT + ct + 1])
                    nc.gpsimd.tensor_scalar(
                        out=s["xn"][:, ct, :], in0=s["x"][:, ct, :],
                        scalar1=ab[:, ct : ct + 1], scalar2=ab[:, CT + ct : CT + ct + 1],
                        op0=Alu.mult, op1=Alu.add,
                    )

            def mm_k(ps, lhs_fn, rhs_fn, nk):
                # contraction helper: DoubleRow pairs (fp8) or singles (bf16)
                for k in range(nk):
                    nc.tensor.matmul(
                        ps[:], lhs_fn(k), rhs_fn(k),
                        start=(k == 0), stop=(k == nk - 1),
                        perf_mode=DR,
                    )

            def ksl(t, k, lo, hi):
                # k-th contraction slab of tile t: [128, KSTEP, lo:hi]
                return t[:, KSTEP * k : KSTEP * (k + 1), lo:hi]

            def t_mm(b, oc, nh):
                # t[:, oc, nh-half] = (M1 xn)[oc-chunk, half] + c1
                s = st[b]
                if "t" not in s:
                    s["t"] = wp.tile([128, CT, HW], CDT, name=f"t{b}", tag="t", bufs=2)
                ps = pp.tile([128, 512], F32, name=f"pt{b}_{oc}_{nh}", tag="mm", bufs=5)
                mm_k(ps,
                     lambda k: ksl(m1, k, oc * 128, (oc + 1) * 128),
                     lambda k: ksl(s["xn"], k, nh * 512, (nh + 1) * 512), NK)
                dst = s["t"][:, oc, nh * 512 : (nh + 1) * 512]
                if (oc + nh) % 2 == 0:
                    nc.scalar.activation(dst, ps[:], Act.Identity, bias=c1[:, oc : oc + 1])
                else:
                    nc.vector.tensor_scalar_add(dst, ps[:], c1[:, oc : oc + 1])

            def ut_mm(b, mb):
                # ut[:, mb, :] = (xn^T M2^T)[mb-block, :]
                s = st[b]
                if "ut" not in s:
                    s["ut"] = wp.tile([128, NBLK, C], CDT, name=f"ut{b}", tag="ut", bufs=2)
                ps = pp.tile([128, 512], F32, name=f"pu{b}_{mb}", tag="mm", bufs=5)
                mm_k(ps,
                     lambda k: ksl(s["xn"], k, mb * 128, (mb + 1) * 128),
                     lambda k: ksl(m2, k, 0, C), NK)
                if mb % 2 == 0:
                    nc.scalar.copy(s["ut"][:, mb, :], ps[:])
                else:
                    nc.vector.tensor_copy(s["ut"][:, mb, :], ps[:])

            def rx_mm(b):
                # general-bias path: rx_t[p, mb] = sum_c wr[c] xn[c, m],
                # then exp-bias slices  INVSQ*rx - SHIFT  (+ bq.bk via host
                # folding into wr? kept here: constant added on host side
                # would shift exp uniformly; folded into SHIFT instead).
                s = st[b]
                s["rxb"] = wp.tile([128, NBLK], F32, name=f"rxb{b}", tag="rxb", bufs=2)
                for mb in range(NBLK):
                    ps = pp.tile([128, 1], F32, name=f"prx{b}_{mb}", tag="gbc", bufs=1)
                    mm_k(ps,
                         lambda k: ksl(s["xn"], k, mb * 128, (mb + 1) * 128),
                         lambda k: ksl(wr, k, 0, 1), NK)
                    nc.vector.tensor_scalar(
                        out=s["rxb"][:, mb : mb + 1], in0=ps[:],
                        scalar1=INVSQ, scalar2=-SHIFT + RXCONST[0],
                        op0=Alu.mult, op1=Alu.add,
                    )

            def sc_mm(b, mb, nh):
                # scores^T tile [m-block, n-half] + exp -> E fp8
                s = st[b]
                if "E" not in s:
                    s["E"] = wp.tile([128, NBLK, HW], CDT, name=f"E{b}", tag="E", bufs=2)
                ps = pp.tile([128, 512], F32, name=f"psc{b}_{mb}_{nh}", tag="mm", bufs=5)
                mm_k(ps,
                     lambda k: ksl(s["t"], k, mb * 128, (mb + 1) * 128),
                     lambda k: ksl(s["xn"], k, nh * 512, (nh + 1) * 512), NK)
                bias = s["rxb"][:, mb : mb + 1] if need_rx else ebias[:, 0:1]
                nc.scalar.activation(
                    s["E"][:, mb, nh * 512 : (nh + 1) * 512], ps[:],
                    Act.Exp, bias=bias, scale=INVSQ,
                )

            def z_mm(b, nh):
                # Z[n] = sum_m E[m, n]: ones-stationary partition reduction,
                # then broadcast Z across partitions and take 1/Z
                s = st[b]
                if "zr" not in s:
                    s["zr"] = wp.tile([128, 2, 512], F32, name=f"zr{b}", tag="zr", bufs=2)
                    s["zsb"] = wp.tile([1, 2, 512], BF16, name=f"zsb{b}", tag="zsb", bufs=2)
                psZ = pp.tile([1, 512], F32, name=f"psz{b}_{nh}", tag="z", bufs=1)
                mm_k(psZ,
                     lambda k: ones_dr[:, :, 0:1] if FP8 else ones_dr[:, 0, 0:1],
                     lambda k: ksl(s["E"], k, nh * 512, (nh + 1) * 512), NM)
                nc.vector.tensor_copy(s["zsb"][:, nh, :], psZ[:])
                psB = pp.tile([128, 512], F32, name=f"psb{b}_{nh}", tag="mm", bufs=5)
                nc.tensor.matmul(psB[:], ones_bc[:], s["zsb"][:, nh, :], start=True, stop=True)
                nc.vector.reciprocal(s["zr"][:, nh, :], psB[:])

            def o_mm(b, ct, nh):
                # out[ct-chunk, nh-half] = outU * zr + b_f + skip, streamed out
                s = st[b]
                out_r = out_d[b].rearrange("(ct p) n -> p ct n", p=128)
                ps = pp.tile([128, 512], F32, name=f"po{b}_{ct}_{nh}", tag="mm", bufs=5)
                mm_k(ps,
                     lambda k: ksl(s["ut"], k, ct * 128, (ct + 1) * 128),
                     lambda k: ksl(s["E"], k, nh * 512, (nh + 1) * 512), NM)
                tmp = wp.tile([128, 512], BF16, name=f"tmp{b}_{ct}_{nh}", tag="tmp", bufs=4)
                nc.vector.tensor_tensor(tmp[:], ps[:], s["zr"][:, nh, :], op=Alu.mult)
                sl = s["x"][:, ct, nh * 512 : (nh + 1) * 512]
                nc.vector.scalar_tensor_tensor(
                    out=sl, in0=tmp[:], scalar=b_f[:, ct : ct + 1], in1=sl,
                    op0=Alu.add, op1=Alu.add,
                )
                nc.sync.dma_start(out_r[:, ct, nh * 512 : (nh + 1) * 512], sl)

            # ---- software-pipelined emission across the two batches ----
            load_x_ct(0, 0); load_x_ct(0, 1)
            m1 = load_weights(0)
            load_x_ct(0, 2); last_x0 = load_x_ct(0, 3)
            m2 = load_weights(1)
            gn_stats(0); gn_apply(0)
            if need_rx:
                rx_mm(0)
            for oc in range(CT):
                t_mm(0, oc, 0); t_mm(0, oc, 1)
            for mb in range(NBLK):
                ut_mm(0, mb)
            for ct in range(CT):
                load_x_ct(1, ct, after=last_x0.ins)
            for mb in range(NBLK):
                sc_mm(0, mb, 0)
            gn_stats(1)
            z_mm(0, 0)
            for mb in range(NBLK):
                sc_mm(0, mb, 1)
            gn_apply(1)
            if need_rx:
                rx_mm(1)
            for ct in range(CT):
                o_mm(0, ct, 0)
            z_mm(0, 1)
            for oc in range(CT):
                t_mm(1, oc, 0); t_mm(1, oc, 1)
            for ct in range(CT):
                o_mm(0, ct, 1)
            for mb in range(NBLK):
                ut_mm(1, mb)
            for mb in range(NBLK):
                sc_mm(1, mb, 0)
            z_mm(1, 0)
            for mb in range(NBLK):
                sc_mm(1, mb, 1)
            for ct in range(CT):
                o_mm(1, ct, 0)
            z_mm(1, 1)
            for ct in range(CT):
                o_mm(1, ct, 1)

    nc.compile()
    return nc


# constant exp-bias addend for the general-bias path (bq.bk term);
# set by _make_in_maps before the program is built
RXCONST = [0.0]


def _get_program(need_rx):
    key = (V2DT, SHIFT, need_rx)
    if key not in _CACHE:
        _CACHE[key] = _build_program(need_rx)
    return _CACHE[key]


def _to_compute(a):
    """Convert host fp32 weights to the matmul compute format."""
    import ml_dtypes
    a = np.ascontiguousarray(a, dtype=np.float32)
    if V2DT == "fp8":
        return np.ascontiguousarray(a.astype(ml_dtypes.float8_e4m3))
    return np.ascontiguousarray(a.astype(ml_dtypes.bfloat16))


def _make_in_maps(x, gamma, beta, w_in, b_in, w_out, b_out):
    x = np.ascontiguousarray(x.reshape(B, C, HW), dtype=np.float32)
    w_in = np.asarray(w_in, dtype=np.float32)
    w_out = np.asarray(w_out, dtype=np.float32)
    b_in = np.asarray(b_in, dtype=np.float32)
    b_out = np.asarray(b_out, dtype=np.float32)
    wq, wk, wv = w_in[0:C], w_in[C : 2 * C], w_in[2 * C : 3 * C]
    bq, bk, bv = b_in[0:C], b_in[C : 2 * C], b_in[2 * C : 3 * C]
    m1 = wq.T @ wk                      # scores = xn^T m1 xn (+ bias terms)
    m2 = w_out @ wv                     # out = m2 xn attn^T + bf
    c1 = wq.T @ bk                      # q-side bias fold (per-channel)
    bf = w_out @ bv + b_out             # exact: softmax rows sum to 1
    wr = wk.T @ bq                      # k-side bias: varies along keys m
    need_rx = bool(np.any(wr) or np.any(bq))
    RXCONST[0] = float(INVSQ * np.dot(bq, bk))

    def cvec(v):
        return np.ascontiguousarray(v.reshape(CT, 128).T, dtype=np.float32)

    consts = {
        "m1t": _to_compute(m1.T),
        "m2t": _to_compute(m2.T),
        "gamma_t": cvec(np.asarray(gamma, dtype=np.float32)),
        "beta_t": cvec(np.asarray(beta, dtype=np.float32)),
        "c1_t": cvec(c1),
        "bf_t": cvec(bf),
        "ind_dn": (np.arange(128)[:, None] // GSIZE == np.arange(GSLOT)[None, :]).astype(np.float32),
        "ind_up": (np.arange(GSLOT)[:, None] == np.arange(128)[None, :] // GSIZE).astype(np.float32),
    }
    if need_rx:
        consts["wr_t"] = _to_compute(wr.reshape(CT, 128).T)
    return need_rx, [
        {"x": x[c * BL : (c + 1) * BL], **consts}
        for c in range(NCORES)
    ]


def run(inputs, trace=False):
    """Run on 8 cores; returns (output [B,C,H,W], BassKernelResults)."""
    from concourse.bass_utils import run_bass_kernel_spmd

    need_rx, in_maps = _make_in_maps(**inputs)
    nc = _get_program(need_rx)
    res = run_bass_kernel_spmd(nc, in_maps, core_ids=list(range(NCORES)), trace=trace)
    out = np.concatenate([res.results[i]["out"] for i in range(NCORES)], axis=0)
    return out.reshape(B, C, H, W).astype(np.float32), res


def kernel(**inputs) -> np.ndarray:
    out, _ = run(inputs)
    return out
